# revision 39
# baseline (speedup 1.0000x reference)
"""Single-head attention (B=4, S=2048, D=E=1024) on 8 TRN2 NeuronCores.

Sharding: data-parallel over (batch, query-half) -> 8 shards. Core c handles
batch b = c//2, query rows h*1024:(h+1)*1024 with h = c%2.

All on-chip compute is done in a "transposed" layout so that every matmul
operand loads naturally (contraction dim on SBUF partitions):
  - host pre-transposes q/k/v to [D, S] and casts to bf16
  - projections produce Q^T [E, SQ], K^T [E, SK] and V [SK, E]
  - scores are computed as S^T [SK, SQ]
  - softmax over SK (partition-tiled) uses exp (no max subtraction: scores
    have std ~1/3, |max| < ~2.5, so exp is safe in fp32/bf16) and a
    ones-vector matmul for the denominator
  - output is produced as O^T [e, sq]; host transposes back

PE efficiency: after tile scheduling fixes the final PE instruction order,
a post-pass marks every matmul whose stationary operand is byte-identical
to the immediately preceding PE instruction's as non-self-loading
(InstMatmult.ldweights = False), so the ~128-cycle LDWEIGHTS is paid once
per run of same-weight matmuls instead of once per matmul.
"""

import sys

if "/opt/trn_rl_repo" not in sys.path:
    sys.path.insert(0, "/opt/trn_rl_repo")

import numpy as np
import ml_dtypes

P = 128
B, S, D, E = 4, 2048, 1024, 1024
SQ = 1024          # query rows per core
SK = 2048          # key/value rows per core (full batch)
SKH = SK // 2      # key rows projected locally before the pair all-gather
DO = D // P        # 8
EO = E // P        # 8
SKT = SK // P      # 16
SKTH = SKH // P    # 8
FD = 512           # matmul moving free dim
NQC = SQ // FD     # 2
SCALE = 1.0 / np.sqrt(np.float32(E))

_NC_CACHE = {}


def _elide_redundant_ldweights(nc, mybir):
    """Post-scheduling pass: walk each basic block's PE instruction stream
    in final order; any matmul whose stationary AP equals the previous PE
    instruction's stationary AP keeps the already-loaded weights
    (ldweights=False). Safe by construction: the comparison is on the
    scheduler-final order, so the weights are guaranteed loaded."""
    n_elided = 0
    for f in nc.m.functions:
        for bb in f.blocks:
            last_key = None
            for inst in bb.instructions:
                if isinstance(inst, mybir.InstLdweights):
                    last_key = repr(inst.ins[0])
                    continue
                if not isinstance(inst, mybir.InstMatmult):
                    continue
                if inst.is_transpose:
                    last_key = None
                    continue
                key = (repr(inst.ins[1]), inst.perf_mode)
                if last_key == key:
                    inst.ldweights = False
                    n_elided += 1
                else:
                    last_key = key
    return n_elided


def build_nc(loop_n=None, collective=True, replicate_n=None, ldw_elide=False,
             alt_weights=False, fp8_scores=True):
    """Build the per-core program.

    loop_n: wrap the body in a hardware For_i loop (bench only; full barrier
    per back-edge, does not work with collectives).
    replicate_n: python-replicate the body N times in one NEFF (bench only;
    iterations overlap like steady-state pipelining, works with collectives).
    alt_weights: chain-outer order (stationary changes every matmul).
    Measured on HW: consecutive matmuls REUSING a stationary tile run at the
    streaming floor (~215ns @ FD=512) while a weight change costs ~+35ns,
    so contraction-outer/chain-inner (False: pairs share the stationary) is
    the better order.
    fp8_scores: store Q^T/K^T in fp8e4m3 and compute the scores matmul with
    DoubleRow perf mode (2 fp8 weights per PE cell -> 256-wide contraction
    per matmul, ~2x effective throughput measured). Also halves the K
    AllGather payload. V/E stay bf16 (fp8 there costs ~3.5% output error;
    fp8 on Q/K costs ~1.2%, inside the 2e-2 budget).
    """
    import concourse.bacc as bacc
    import concourse.mybir as mybir
    import concourse.tile as tile
    from concourse.bass import ts
    from contextlib import nullcontext

    bf16 = mybir.dt.bfloat16
    f32 = mybir.dt.float32
    fp8 = mybir.dt.float8e4
    DR = mybir.MatmulPerfMode.DoubleRow
    kdt = fp8 if fp8_scores else bf16
    Exp = mybir.ActivationFunctionType.Exp
    mult = mybir.AluOpType.mult

    nc = bacc.Bacc("TRN2", target_bir_lowering=False, debug=False, num_devices=8)

    qT = nc.dram_tensor("qT", [D, SQ], bf16, kind="ExternalInput").ap()
    kT = nc.dram_tensor("kT", [D, SK], bf16, kind="ExternalInput").ap()
    vT = nc.dram_tensor("vT", [D, SK], bf16, kind="ExternalInput").ap()
    wq = nc.dram_tensor("wq", [D, E], bf16, kind="ExternalInput").ap()
    wk = nc.dram_tensor("wk", [D, E], bf16, kind="ExternalInput").ap()
    wv = nc.dram_tensor("wv", [D, E], bf16, kind="ExternalInput").ap()
    if replicate_n:
        # per-replica output slices so neuronx-cc can't dead-store-eliminate
        # the earlier replicas (bench-only shape)
        outT_full = nc.dram_tensor(
            "outT", [replicate_n, E, SQ], f32, kind="ExternalOutput").ap()
    else:
        outT = nc.dram_tensor("outT", [E, SQ], f32, kind="ExternalOutput").ap()

    def mm_order(n_contract, n_chain):
        """(contraction_idx, chain_idx) emission order for a matmul block.

        alt_weights: chain-outer/contraction-inner -> the stationary tile
        (indexed by the contraction idx) differs between every pair of
        consecutive matmuls, letting the PE overlap each LDWEIGHTS with the
        previous matmul. Otherwise contraction-outer (stationary shared by
        consecutive chain matmuls)."""
        if alt_weights:
            for c in range(n_chain):
                for k in range(n_contract):
                    yield k, c
        else:
            for k in range(n_contract):
                for c in range(n_chain):
                    yield k, c

    qT3 = qT.rearrange("(o p) s -> p o s", p=P)
    kT3 = kT.rearrange("(o p) s -> p o s", p=P)
    vT3 = vT.rearrange("(o p) s -> p o s", p=P)
    wq3 = wq.rearrange("(o p) e -> p o e", p=P)
    wk3 = wk.rearrange("(o p) e -> p o e", p=P)
    wv3 = wv.rearrange("(o p) e -> p o e", p=P)

    with tile.TileContext(nc) as tc:
        with tc.tile_pool(name="persist", bufs=1) as persist, \
             tc.tile_pool(name="epool", bufs=2) as epool, \
             tc.tile_pool(name="wpool", bufs=2) as wpool, \
             tc.tile_pool(name="stream", bufs=4) as stream, \
             tc.tile_pool(name="misc", bufs=1) as misc, \
             tc.tile_pool(name="ostage", bufs=2) as ostage, \
             tc.tile_pool(name="dram", bufs=2, space="DRAM") as dram, \
             tc.tile_pool(name="psum", bufs=7, space="PSUM") as psum, \
             (tc.For_i(0, loop_n, 1) if loop_n else nullcontext()):

            for _rep in range(replicate_n or 1):
                if replicate_n:
                    outT = outT_full[_rep]

                # ---- persistent on-chip tensors -------------------------------
                # E_s is double-buffered (epool) so the next body's scores/exp
                # can land while this body's AV matmuls still read E_s.
                V_s = persist.tile([P, SKT, E], bf16, tag="V")     # V[sk, e]
                E_s = epool.tile([P, SKT, SQ], bf16, tag="EW")     # exp(S^T)[sk, sq]
                if fp8_scores:
                    # fp8 DoubleRow layout: e-tile et -> (group eg, half khi)
                    # with et = 2*eg + khi; contraction pairs (partition, khi).
                    KT_s = persist.tile([P, EO // 2, 2, SK], fp8, tag="KT")
                    QT_s = persist.tile([P, EO // 2, 2, SQ], fp8, tag="QT")

                    def kt_dst(et, cols):
                        return KT_s[:, et // 2, et % 2, cols]

                    def qt_dst(et, cols):
                        return QT_s[:, et // 2, et % 2, cols]
                else:
                    KT_s = persist.tile([P, EO, SK], bf16, tag="KT")
                    QT_s = persist.tile([P, EO, SQ], bf16, tag="QT")

                    def kt_dst(et, cols):
                        return KT_s[:, et, cols]

                    def qt_dst(et, cols):
                        return QT_s[:, et, cols]

                # [P, P] of ones: ones.T @ E gives the column sums replicated on
                # every output partition -> softmax denominator pre-broadcast.
                ones = misc.tile([P, P], bf16, tag="ones")
                nc.any.memset(ones[:], 1.0)

                def emit_q_proj():
                    # ---- Q^T = (q @ Wq)^T, [e, sq] ----------------------------
                    wq_s = wpool.tile([P, DO, E], bf16, tag="w")
                    nc.sync.dma_start(wq_s[:], wq3)
                    qcs = []
                    for ci in range(NQC):
                        qc = stream.tile([P, DO, FD], bf16, tag="xtc",
                                         name=f"qc{ci}")
                        nc.sync.dma_start(qc[:], qT3[:, :, ts(ci, FD)])
                        qcs.append(qc)
                    for et in range(EO):
                        pss = [psum.tile([P, FD], f32, tag="mm", name=f"ps{ci}")
                               for ci in range(NQC)]
                        for do, ci in mm_order(DO, NQC):
                            nc.tensor.matmul(
                                pss[ci][:], wq_s[:, do, ts(et, P)],
                                qcs[ci][:, do, :],
                                start=(do == 0), stop=(do == DO - 1),
                            )
                        for ci in range(NQC):
                            nc.any.tensor_copy(qt_dst(et, ts(ci, FD)), pss[ci][:])

                if not collective:
                    # ---- V = v @ Wv (full), natural [sk, e] -------------------
                    wv_s = wpool.tile([P, DO, E], bf16, tag="w")
                    nc.sync.dma_start(wv_s[:], wv3)
                    for skt in range(SKT):
                        vt = stream.tile([P, DO, P], bf16, tag="xtv")
                        nc.sync.dma_start(vt[:], vT3[:, :, ts(skt, P)])
                        pss = [psum.tile([P, FD], f32, tag="mm", name=f"ps{c}") for c in range(E // FD)]
                        for do, c in mm_order(DO, E // FD):
                            nc.tensor.matmul(
                                pss[c][:], vt[:, do, :], wv_s[:, do, ts(c, FD)],
                                start=(do == 0), stop=(do == DO - 1),
                            )
                        for c in range(E // FD):
                            nc.any.tensor_copy(V_s[:, skt, ts(c, FD)], pss[c][:])

                    # ---- K^T = (k @ Wk)^T (full), [e, sk] ---------------------
                    wk_s = wpool.tile([P, DO, E], bf16, tag="w")
                    nc.sync.dma_start(wk_s[:], wk3)
                    for cp in range(SK // FD // 2):
                        kcs = []
                        for ci in range(2):
                            kc = stream.tile([P, DO, FD], bf16, tag="xtc",
                                             name=f"kc{ci}")
                            nc.sync.dma_start(
                                kc[:], kT3[:, :, ts(2 * cp + ci, FD)])
                            kcs.append(kc)
                        for et in range(EO):
                            pss = [psum.tile([P, FD], f32, tag="mm", name=f"ps{ci}")
                                   for ci in range(2)]
                            for do, ci in mm_order(DO, 2):
                                nc.tensor.matmul(
                                    pss[ci][:], wk_s[:, do, ts(et, P)],
                                    kcs[ci][:, do, :],
                                    start=(do == 0), stop=(do == DO - 1),
                                )
                            for ci in range(2):
                                nc.any.tensor_copy(
                                    kt_dst(et, ts(2 * cp + ci, FD)), pss[ci][:])
                else:
                    # The host passes this core's key-half in the FIRST half of
                    # kT/vT inputs (columns 0:1024); the other pair member gets
                    # the complementary half. Project columns 0:1024 only, ship
                    # through pair AllGathers (K first so it lands before the
                    # scores need it; V gathered second -- its consumer, the AV
                    # matmul, runs much later, so the V gather hides behind the
                    # Q projection and the whole scores phase).
                    kb_k = dram.tile([E, SKH], kdt, tag="kbk")
                    gb_k = dram.tile([2, E, SKH], kdt, tag="gbk")
                    kb_v = dram.tile([SKH, E], bf16, tag="kbv")
                    gb_v = dram.tile([2, SKH, E], bf16, tag="gbv")

                    # ---- K^T half: [e, 0:1024] --------------------------------
                    wk_s = wpool.tile([P, DO, E], bf16, tag="w")
                    nc.sync.dma_start(wk_s[:], wk3)
                    kb_k3 = kb_k.rearrange("(o p) s -> p o s", p=P)
                    for c in range(SKH // FD):
                        kc = stream.tile([P, DO, FD], bf16, tag="xtc")
                        nc.sync.dma_start(kc[:], kT3[:, :, ts(c, FD)])
                        for et in range(EO):
                            ps = psum.tile([P, FD], f32, tag="mm")
                            for do in range(DO):
                                nc.tensor.matmul(
                                    ps[:], wk_s[:, do, ts(et, P)], kc[:, do, :],
                                    start=(do == 0), stop=(do == DO - 1),
                                )
                            kst = stream.tile([P, FD], kdt, tag="kst8")
                            nc.vector.tensor_copy(kst[:], ps[:])
                            nc.sync.dma_start(kb_k3[:, et, ts(c, FD)], kst[:])

                    nc.gpsimd.collective_compute(
                        "AllGather",
                        mybir.AluOpType.bypass,
                        replica_groups=[[0, 1], [2, 3], [4, 5], [6, 7]],
                        ins=[kb_k.opt()],
                        outs=[gb_k.opt()],
                    )

                    # Q projection here: independent PE work that hides the
                    # K AllGather latency before the scores need K.
                    emit_q_proj()

                    # ---- V half: rows 0:1024, natural [skh, e] ----------------
                    wv_s = wpool.tile([P, DO, E], bf16, tag="w")
                    nc.sync.dma_start(wv_s[:], wv3)
                    kb_v3 = kb_v.rearrange("(t p) e -> p t e", p=P)  # [p, SKTH, e]
                    for skt in range(SKTH):
                        vt = stream.tile([P, DO, P], bf16, tag="xtv")
                        nc.sync.dma_start(vt[:], vT3[:, :, ts(skt, P)])
                        for c in range(E // FD):
                            ps = psum.tile([P, FD], f32, tag="mm")
                            for do in range(DO):
                                nc.tensor.matmul(
                                    ps[:], vt[:, do, :], wv_s[:, do, ts(c, FD)],
                                    start=(do == 0), stop=(do == DO - 1),
                                )
                            vst = stream.tile([P, FD], bf16, tag="kstv")
                            nc.vector.tensor_copy(vst[:], ps[:])
                            nc.sync.dma_start(kb_v3[:, skt, ts(c, FD)], vst[:])

                    nc.gpsimd.collective_compute(
                        "AllGather",
                        mybir.AluOpType.bypass,
                        replica_groups=[[0, 1], [2, 3], [4, 5], [6, 7]],
                        ins=[kb_v.opt()],
                        outs=[gb_v.opt()],
                    )

                if not collective:
                    emit_q_proj()
                else:
                    # unpack gathered pair results into K^T [e, sk] and V [sk, e]
                    # rank r of the pair contributed global key rows r*1024:+1024;
                    # the host permutes its kT/vT inputs so that "column block 0"
                    # of each core is that core's own half -> gathered slot r
                    # corresponds to global rows r*1024. Unpack in quarter-SKH
                    # chunks so the first scores tiles can start while later
                    # chunks are still in flight.
                    for r in range(2):
                        g_k3 = gb_k[r].rearrange("(o p) s -> p o s", p=P)
                        for half in range(2):
                            colslice = slice(r * SKH + half * FD,
                                             r * SKH + (half + 1) * FD)
                            if fp8_scores:
                                kdst = KT_s[:, :, :, colslice]
                            else:
                                kdst = KT_s[:, :, colslice]
                            nc.sync.dma_start(kdst, g_k3[:, :, ts(half, FD)])
                        g_v3 = gb_v[r].rearrange("(t p) e -> p t e", p=P)
                        for half in range(2):
                            nc.sync.dma_start(
                                V_s[:, r * SKTH + half * (SKTH // 2):
                                    r * SKTH + (half + 1) * (SKTH // 2), :],
                                g_v3[:, half * (SKTH // 2):
                                     (half + 1) * (SKTH // 2), :])

                # ---- E = exp(scale * S^T),  S^T[sk, sq] = K Q^T ---------------
                # c-inner so consecutive matmuls share the stationary lhsT
                for skt in range(SKT):
                    pss = [psum.tile([P, FD], f32, tag="mm", name=f"ps{c}") for c in range(NQC)]
                    if fp8_scores:
                        for eg, c in mm_order(EO // 2, NQC):
                            nc.tensor.matmul(
                                pss[c][:], KT_s[:, eg, :, ts(skt, P)],
                                QT_s[:, eg, :, ts(c, FD)],
                                start=(eg == 0), stop=(eg == EO // 2 - 1),
                                perf_mode=DR,
                            )
                    else:
                        for et, c in mm_order(EO, NQC):
                            nc.tensor.matmul(
                                pss[c][:], KT_s[:, et, ts(skt, P)], QT_s[:, et, ts(c, FD)],
                                start=(et == 0), stop=(et == EO - 1),
                            )
                    for c in range(NQC):
                        nc.scalar.activation(
                            E_s[:, skt, ts(c, FD)], pss[c][:], Exp, scale=float(SCALE)
                        )

                # ---- softmax denominator: rden[:, sq] = 1 / sum_sk E[sk, sq] --
                # ones.T @ E replicates the column sum on all 128 partitions.
                rden = misc.tile([P, SQ], f32, tag="rden")
                for c in range(NQC):
                    psd = psum.tile([P, FD], f32, tag="den", bufs=1)
                    for skt in range(SKT):
                        nc.tensor.matmul(
                            psd[:], ones[:, :], E_s[:, skt, ts(c, FD)],
                            start=(skt == 0), stop=(skt == SKT - 1),
                        )
                    nc.vector.reciprocal(rden[:, ts(c, FD)], psd[:])

                # ---- O^T[e, sq] = V^T E, then normalize and store -------------
                for et in range(EO):
                    pss = [psum.tile([P, FD], f32, tag="mm", name=f"ps{c}") for c in range(NQC)]
                    for skt, c in mm_order(SKT, NQC):
                        nc.tensor.matmul(
                            pss[c][:], V_s[:, skt, ts(et, P)], E_s[:, skt, ts(c, FD)],
                            start=(skt == 0), stop=(skt == SKT - 1),
                        )
                    for c in range(NQC):
                        ot = ostage.tile([P, FD], f32, tag="ot")
                        nc.vector.tensor_tensor(
                            ot[:], pss[c][:], rden[:, ts(c, FD)], mult
                        )
                        nc.sync.dma_start(outT[ts(et, P), ts(c, FD)], ot[:])

    if ldw_elide:
        n = _elide_redundant_ldweights(nc, mybir)
        print(f"ldweights elided: {n}")

    nc.compile()
    return nc


def get_nc():
    if "nc" not in _NC_CACHE:
        _NC_CACHE["nc"] = build_nc()
    return _NC_CACHE["nc"]


def make_in_maps(q, k, v, W_q, W_k, W_v, collective=True):
    bf = ml_dtypes.bfloat16
    wq = np.ascontiguousarray(W_q.astype(bf))
    wk = np.ascontiguousarray(W_k.astype(bf))
    wv = np.ascontiguousarray(W_v.astype(bf))
    kTb = [np.ascontiguousarray(k[b].astype(bf).T) for b in range(B)]
    vTb = [np.ascontiguousarray(v[b].astype(bf).T) for b in range(B)]
    in_maps = []
    for c in range(8):
        b, h = c // 2, c % 2
        qTc = np.ascontiguousarray(q[b, h * SQ:(h + 1) * SQ, :].astype(bf).T)
        kTc, vTc = kTb[b], vTb[b]
        if collective and h == 1:
            # odd core projects the second key-half: swap halves so its own
            # half sits in columns 0:1024 (the projected range)
            kTc = np.ascontiguousarray(
                np.concatenate([kTc[:, SKH:], kTc[:, :SKH]], axis=1))
            vTc = np.ascontiguousarray(
                np.concatenate([vTc[:, SKH:], vTc[:, :SKH]], axis=1))
        in_maps.append({
            "qT": qTc, "kT": kTc, "vT": vTc,
            "wq": wq, "wk": wk, "wv": wv,
        })
    return in_maps


def kernel(q, k, v, W_q, W_k, W_v):
    from concourse import bass_utils

    q, k, v = np.asarray(q), np.asarray(k), np.asarray(v)
    W_q, W_k, W_v = np.asarray(W_q), np.asarray(W_k), np.asarray(W_v)
    nc = get_nc()
    in_maps = make_in_maps(q, k, v, W_q, W_k, W_v)
    res = bass_utils.run_bass_kernel_spmd(nc, in_maps, core_ids=list(range(8)))
    out = np.empty((B, S, E), dtype=np.float32)
    for c in range(8):
        b, h = c // 2, c % 2
        out[b, h * SQ:(h + 1) * SQ, :] = res.results[c]["outT"].T
    return out


# revision 47
# speedup vs baseline: 1.0027x; 1.0027x over previous
"""Single-head attention (B=4, S=2048, D=E=1024) on 8 TRN2 NeuronCores.

Sharding: data-parallel over (batch, query-half) -> 8 shards. Core c handles
batch b = c//2, query rows h*1024:(h+1)*1024 with h = c%2.

All on-chip compute is done in a "transposed" layout so that every matmul
operand loads naturally (contraction dim on SBUF partitions):
  - host pre-transposes q/k/v to [D, S] and casts to bf16
  - projections produce Q^T [E, SQ], K^T [E, SK] and V [SK, E]
  - scores are computed as S^T [SK, SQ]
  - softmax over SK (partition-tiled) uses exp (no max subtraction: scores
    have std ~1/3, |max| < ~2.5, so exp is safe in fp32/bf16) and a
    ones-vector matmul for the denominator
  - output is produced as O^T [e, sq]; host transposes back

Performance structure (HW-measured, replicate-delta method):
  - pair sharding of the K/V projections: each core projects only its key
    half; halves are exchanged with two pair AllGathers (K gathered early,
    hidden behind the Q/V projections; V gathered late, hidden behind the
    whole scores phase). DRAM bounce tiles are double-buffered so the
    collectives also overlap across pipelined iterations.
  - the scores matmul runs in fp8e4m3 with DoubleRow perf mode (2 fp8
    weights per PE cell, 256-wide contraction per matmul, ~2x measured
    throughput); Q^T/K^T are stored fp8 (also halving the K AllGather).
    V and exp(S) stay bf16: fp8 there injects ~3.5% output error (the
    softmax-weighted sum of near-random V does not average quantization
    noise down), while fp8 on Q/K costs ~1.2%, within the 2e-2 budget.
  - consecutive matmuls share their stationary tile in pairs (c-inner
    loops): a same-weight matmul runs at the ~215ns streaming floor while
    a weight change costs ~+35ns (walrus emits LDWEIGHTS per matmul and no
    flag elides it; reuse-adjacency is the only lever).
  - E_s is double-buffered so the next iteration's exp results can land
    while this iteration's AV matmuls still read the previous E_s.
"""

import sys

if "/opt/trn_rl_repo" not in sys.path:
    sys.path.insert(0, "/opt/trn_rl_repo")

import numpy as np
import ml_dtypes

P = 128
B, S, D, E = 4, 2048, 1024, 1024
SQ = 1024          # query rows per core
SK = 2048          # key/value rows per core (full batch)
SKH = SK // 2      # key rows projected locally before the pair all-gather
DO = D // P        # 8
EO = E // P        # 8
SKT = SK // P      # 16
SKTH = SKH // P    # 8
FD = 512           # matmul moving free dim
NQC = SQ // FD     # 2
SCALE = 1.0 / np.sqrt(np.float32(E))

_NC_CACHE = {}


def _elide_redundant_ldweights(nc, mybir):
    """Post-scheduling pass: walk each basic block's PE instruction stream
    in final order; any matmul whose stationary AP equals the previous PE
    instruction's stationary AP keeps the already-loaded weights
    (ldweights=False). Safe by construction: the comparison is on the
    scheduler-final order, so the weights are guaranteed loaded."""
    n_elided = 0
    for f in nc.m.functions:
        for bb in f.blocks:
            last_key = None
            for inst in bb.instructions:
                if isinstance(inst, mybir.InstLdweights):
                    last_key = repr(inst.ins[0])
                    continue
                if not isinstance(inst, mybir.InstMatmult):
                    continue
                if inst.is_transpose:
                    last_key = None
                    continue
                key = (repr(inst.ins[1]), inst.perf_mode)
                if last_key == key:
                    inst.ldweights = False
                    n_elided += 1
                else:
                    last_key = key
    return n_elided


def build_nc(loop_n=None, collective=True, replicate_n=None, ldw_elide=False,
             alt_weights=False, fp8_scores=True):
    """Build the per-core program.

    loop_n: wrap the body in a hardware For_i loop (bench only; full barrier
    per back-edge, does not work with collectives).
    replicate_n: python-replicate the body N times in one NEFF (bench only;
    iterations overlap like steady-state pipelining, works with collectives).
    alt_weights: chain-outer order (stationary changes every matmul).
    Measured on HW: consecutive matmuls REUSING a stationary tile run at the
    streaming floor (~215ns @ FD=512) while a weight change costs ~+35ns,
    so contraction-outer/chain-inner (False: pairs share the stationary) is
    the better order.
    fp8_scores: store Q^T/K^T in fp8e4m3 and compute the scores matmul with
    DoubleRow perf mode (2 fp8 weights per PE cell -> 256-wide contraction
    per matmul, ~2x effective throughput measured). Also halves the K
    AllGather payload. V/E stay bf16 (fp8 there costs ~3.5% output error;
    fp8 on Q/K costs ~1.2%, inside the 2e-2 budget).
    """
    import concourse.bacc as bacc
    import concourse.mybir as mybir
    import concourse.tile as tile
    from concourse import bass_isa
    from concourse.bass import ts
    from contextlib import nullcontext

    bf16 = mybir.dt.bfloat16
    f32 = mybir.dt.float32
    fp8 = mybir.dt.float8e4
    DR = mybir.MatmulPerfMode.DoubleRow
    kdt = fp8 if fp8_scores else bf16
    Exp = mybir.ActivationFunctionType.Exp
    mult = mybir.AluOpType.mult
    add = mybir.AluOpType.add

    nc = bacc.Bacc("TRN2", target_bir_lowering=False, debug=False, num_devices=8)

    qT = nc.dram_tensor("qT", [D, SQ], bf16, kind="ExternalInput").ap()
    kT = nc.dram_tensor("kT", [D, SK], bf16, kind="ExternalInput").ap()
    vT = nc.dram_tensor("vT", [D, SK], bf16, kind="ExternalInput").ap()
    wq = nc.dram_tensor("wq", [D, E], bf16, kind="ExternalInput").ap()
    wk = nc.dram_tensor("wk", [D, E], bf16, kind="ExternalInput").ap()
    wv = nc.dram_tensor("wv", [D, E], bf16, kind="ExternalInput").ap()
    if replicate_n:
        # per-replica output slices so neuronx-cc can't dead-store-eliminate
        # the earlier replicas (bench-only shape)
        outT_full = nc.dram_tensor(
            "outT", [replicate_n, E, SQ], f32, kind="ExternalOutput").ap()
    else:
        outT = nc.dram_tensor("outT", [E, SQ], f32, kind="ExternalOutput").ap()

    def mm_order(n_contract, n_chain):
        """(contraction_idx, chain_idx) emission order for a matmul block.

        alt_weights: chain-outer/contraction-inner -> the stationary tile
        (indexed by the contraction idx) differs between every pair of
        consecutive matmuls, letting the PE overlap each LDWEIGHTS with the
        previous matmul. Otherwise contraction-outer (stationary shared by
        consecutive chain matmuls)."""
        if alt_weights:
            for c in range(n_chain):
                for k in range(n_contract):
                    yield k, c
        else:
            for k in range(n_contract):
                for c in range(n_chain):
                    yield k, c

    qT3 = qT.rearrange("(o p) s -> p o s", p=P)
    kT3 = kT.rearrange("(o p) s -> p o s", p=P)
    vT3 = vT.rearrange("(o p) s -> p o s", p=P)
    wq3 = wq.rearrange("(o p) e -> p o e", p=P)
    wk3 = wk.rearrange("(o p) e -> p o e", p=P)
    wv3 = wv.rearrange("(o p) e -> p o e", p=P)

    with tile.TileContext(nc) as tc:
        with tc.tile_pool(name="persist", bufs=1) as persist, \
             tc.tile_pool(name="epool", bufs=2) as epool, \
             tc.tile_pool(name="wpool", bufs=2) as wpool, \
             tc.tile_pool(name="stream", bufs=3) as stream, \
             tc.tile_pool(name="misc", bufs=1) as misc, \
             tc.tile_pool(name="ostage", bufs=2) as ostage, \
             tc.tile_pool(name="dram", bufs=2, space="DRAM") as dram, \
             tc.tile_pool(name="psum", bufs=6, space="PSUM") as psum, \
             (tc.For_i(0, loop_n, 1) if loop_n else nullcontext()):

            for _rep in range(replicate_n or 1):
                if replicate_n:
                    outT = outT_full[_rep]

                # ---- persistent on-chip tensors -------------------------------
                # E_s is double-buffered (epool) so the next body's scores/exp
                # can land while this body's AV matmuls still read E_s.
                V_s = persist.tile([P, SKT, E], bf16, tag="V")     # V[sk, e]
                E_s = epool.tile([P, SKT, SQ], bf16, tag="EW")     # exp(S^T)[sk, sq]
                if fp8_scores:
                    # fp8 DoubleRow layout: e-tile et -> (group eg, half khi)
                    # with et = 2*eg + khi; contraction pairs (partition, khi).
                    KT_s = persist.tile([P, EO // 2, 2, SK], fp8, tag="KT")
                    QT_s = persist.tile([P, EO // 2, 2, SQ], fp8, tag="QT")

                    def kt_dst(et, cols):
                        return KT_s[:, et // 2, et % 2, cols]

                    def qt_dst(et, cols):
                        return QT_s[:, et // 2, et % 2, cols]
                else:
                    KT_s = persist.tile([P, EO, SK], bf16, tag="KT")
                    QT_s = persist.tile([P, EO, SQ], bf16, tag="QT")

                    def kt_dst(et, cols):
                        return KT_s[:, et, cols]

                    def qt_dst(et, cols):
                        return QT_s[:, et, cols]

                # f32 running column-sum of exp(S^T) over the sk tiles,
                # accumulated on the (otherwise slack) vector engine during
                # the scores phase; the cross-partition sum runs on the idle
                # GpSimd engine, freeing the PE of all denominator matmuls.
                esum = misc.tile([P, SQ], f32, tag="esum")

                def emit_q_proj():
                    # ---- Q^T = (q @ Wq)^T, [e, sq] ----------------------------
                    wq_s = wpool.tile([P, DO, E], bf16, tag="w")
                    nc.sync.dma_start(wq_s[:], wq3)
                    qcs = []
                    for ci in range(NQC):
                        qc = stream.tile([P, DO, FD], bf16, tag="xtc",
                                         name=f"qc{ci}")
                        nc.sync.dma_start(qc[:], qT3[:, :, ts(ci, FD)])
                        qcs.append(qc)
                    for et in range(EO):
                        pss = [psum.tile([P, FD], f32, tag="mm", name=f"ps{ci}")
                               for ci in range(NQC)]
                        for do, ci in mm_order(DO, NQC):
                            nc.tensor.matmul(
                                pss[ci][:], wq_s[:, do, ts(et, P)],
                                qcs[ci][:, do, :],
                                start=(do == 0), stop=(do == DO - 1),
                            )
                        for ci in range(NQC):
                            nc.any.tensor_copy(qt_dst(et, ts(ci, FD)), pss[ci][:])

                if not collective:
                    # ---- V = v @ Wv (full), natural [sk, e] -------------------
                    wv_s = wpool.tile([P, DO, E], bf16, tag="w")
                    nc.sync.dma_start(wv_s[:], wv3)
                    for skt in range(SKT):
                        vt = stream.tile([P, DO, P], bf16, tag="xtv")
                        nc.sync.dma_start(vt[:], vT3[:, :, ts(skt, P)])
                        pss = [psum.tile([P, FD], f32, tag="mm", name=f"ps{c}") for c in range(E // FD)]
                        for do, c in mm_order(DO, E // FD):
                            nc.tensor.matmul(
                                pss[c][:], vt[:, do, :], wv_s[:, do, ts(c, FD)],
                                start=(do == 0), stop=(do == DO - 1),
                            )
                        for c in range(E // FD):
                            nc.any.tensor_copy(V_s[:, skt, ts(c, FD)], pss[c][:])

                    # ---- K^T = (k @ Wk)^T (full), [e, sk] ---------------------
                    wk_s = wpool.tile([P, DO, E], bf16, tag="w")
                    nc.sync.dma_start(wk_s[:], wk3)
                    for cp in range(SK // FD // 2):
                        kcs = []
                        for ci in range(2):
                            kc = stream.tile([P, DO, FD], bf16, tag="xtc",
                                             name=f"kc{ci}")
                            nc.sync.dma_start(
                                kc[:], kT3[:, :, ts(2 * cp + ci, FD)])
                            kcs.append(kc)
                        for et in range(EO):
                            pss = [psum.tile([P, FD], f32, tag="mm", name=f"ps{ci}")
                                   for ci in range(2)]
                            for do, ci in mm_order(DO, 2):
                                nc.tensor.matmul(
                                    pss[ci][:], wk_s[:, do, ts(et, P)],
                                    kcs[ci][:, do, :],
                                    start=(do == 0), stop=(do == DO - 1),
                                )
                            for ci in range(2):
                                nc.any.tensor_copy(
                                    kt_dst(et, ts(2 * cp + ci, FD)), pss[ci][:])
                else:
                    # The host passes this core's key-half in the FIRST half of
                    # kT/vT inputs (columns 0:1024); the other pair member gets
                    # the complementary half. Project columns 0:1024 only, ship
                    # through pair AllGathers (K first so it lands before the
                    # scores need it; V gathered second -- its consumer, the AV
                    # matmul, runs much later, so the V gather hides behind the
                    # Q projection and the whole scores phase).
                    kb_k = dram.tile([E, SKH], kdt, tag="kbk")
                    gb_k = dram.tile([2, E, SKH], kdt, tag="gbk")
                    kb_v = dram.tile([SKH, E], bf16, tag="kbv")
                    gb_v = dram.tile([2, SKH, E], bf16, tag="gbv")

                    # ---- K^T half: [e, 0:1024] --------------------------------
                    wk_s = wpool.tile([P, DO, E], bf16, tag="w")
                    nc.sync.dma_start(wk_s[:], wk3)
                    kb_k3 = kb_k.rearrange("(o p) s -> p o s", p=P)
                    for c in range(SKH // FD):
                        kc = stream.tile([P, DO, FD], bf16, tag="xtc")
                        nc.sync.dma_start(kc[:], kT3[:, :, ts(c, FD)])
                        for et in range(EO):
                            ps = psum.tile([P, FD], f32, tag="mm")
                            for do in range(DO):
                                nc.tensor.matmul(
                                    ps[:], wk_s[:, do, ts(et, P)], kc[:, do, :],
                                    start=(do == 0), stop=(do == DO - 1),
                                )
                            kst = stream.tile([P, FD], kdt, tag="kst8")
                            nc.vector.tensor_copy(kst[:], ps[:])
                            nc.sync.dma_start(kb_k3[:, et, ts(c, FD)], kst[:])

                    nc.gpsimd.collective_compute(
                        "AllGather",
                        mybir.AluOpType.bypass,
                        replica_groups=[[0, 1], [2, 3], [4, 5], [6, 7]],
                        ins=[kb_k.opt()],
                        outs=[gb_k.opt()],
                    )

                    # Q projection here: independent PE work that hides the
                    # K AllGather latency before the scores need K.
                    emit_q_proj()

                    # ---- V half: rows 0:1024, natural [skh, e] ----------------
                    wv_s = wpool.tile([P, DO, E], bf16, tag="w")
                    nc.sync.dma_start(wv_s[:], wv3)
                    kb_v3 = kb_v.rearrange("(t p) e -> p t e", p=P)  # [p, SKTH, e]
                    for skt in range(SKTH):
                        vt = stream.tile([P, DO, P], bf16, tag="xtv")
                        nc.sync.dma_start(vt[:], vT3[:, :, ts(skt, P)])
                        for c in range(E // FD):
                            ps = psum.tile([P, FD], f32, tag="mm")
                            for do in range(DO):
                                nc.tensor.matmul(
                                    ps[:], vt[:, do, :], wv_s[:, do, ts(c, FD)],
                                    start=(do == 0), stop=(do == DO - 1),
                                )
                            vst = stream.tile([P, FD], bf16, tag="kstv")
                            nc.vector.tensor_copy(vst[:], ps[:])
                            nc.sync.dma_start(kb_v3[:, skt, ts(c, FD)], vst[:])

                    nc.gpsimd.collective_compute(
                        "AllGather",
                        mybir.AluOpType.bypass,
                        replica_groups=[[0, 1], [2, 3], [4, 5], [6, 7]],
                        ins=[kb_v.opt()],
                        outs=[gb_v.opt()],
                    )

                if not collective:
                    emit_q_proj()
                else:
                    # unpack gathered pair results into K^T [e, sk] and V [sk, e]
                    # rank r of the pair contributed global key rows r*1024:+1024;
                    # the host permutes its kT/vT inputs so that "column block 0"
                    # of each core is that core's own half -> gathered slot r
                    # corresponds to global rows r*1024. Unpack in quarter-SKH
                    # chunks so the first scores tiles can start while later
                    # chunks are still in flight.
                    for r in range(2):
                        g_k3 = gb_k[r].rearrange("(o p) s -> p o s", p=P)
                        for half in range(2):
                            colslice = slice(r * SKH + half * FD,
                                             r * SKH + (half + 1) * FD)
                            if fp8_scores:
                                kdst = KT_s[:, :, :, colslice]
                            else:
                                kdst = KT_s[:, :, colslice]
                            nc.sync.dma_start(kdst, g_k3[:, :, ts(half, FD)])
                        g_v3 = gb_v[r].rearrange("(t p) e -> p t e", p=P)
                        for half in range(2):
                            nc.sync.dma_start(
                                V_s[:, r * SKTH + half * (SKTH // 2):
                                    r * SKTH + (half + 1) * (SKTH // 2), :],
                                g_v3[:, half * (SKTH // 2):
                                     (half + 1) * (SKTH // 2), :])

                # ---- E = exp(scale * S^T),  S^T[sk, sq] = K Q^T ---------------
                # c-inner so consecutive matmuls share the stationary lhsT
                for skt in range(SKT):
                    pss = [psum.tile([P, FD], f32, tag="mm", name=f"ps{c}") for c in range(NQC)]
                    if fp8_scores:
                        for eg, c in mm_order(EO // 2, NQC):
                            nc.tensor.matmul(
                                pss[c][:], KT_s[:, eg, :, ts(skt, P)],
                                QT_s[:, eg, :, ts(c, FD)],
                                start=(eg == 0), stop=(eg == EO // 2 - 1),
                                perf_mode=DR,
                            )
                    else:
                        for et, c in mm_order(EO, NQC):
                            nc.tensor.matmul(
                                pss[c][:], KT_s[:, et, ts(skt, P)], QT_s[:, et, ts(c, FD)],
                                start=(et == 0), stop=(et == EO - 1),
                            )
                    for c in range(NQC):
                        nc.scalar.activation(
                            E_s[:, skt, ts(c, FD)], pss[c][:], Exp, scale=float(SCALE)
                        )
                    if skt == 0:
                        nc.vector.tensor_copy(esum[:], E_s[:, 0, :])
                    else:
                        nc.vector.tensor_tensor(
                            esum[:], esum[:], E_s[:, skt, :], add)

                # ---- softmax denominator: rden[:, sq] = 1 / sum_sk E[sk, sq] --
                # esum holds the per-partition partial sums; the GpSimd
                # partition all-reduce replicates the full column sum on every
                # partition (same layout the ones-matmul used to produce).
                rdsum = misc.tile([P, SQ], f32, tag="rdsum")
                nc.gpsimd.partition_all_reduce(
                    rdsum[:], esum[:], 128, bass_isa.ReduceOp.add)
                rden = misc.tile([P, SQ], f32, tag="rden")
                nc.vector.reciprocal(rden[:], rdsum[:])

                # ---- O^T[e, sq] = V^T E, then normalize and store -------------
                for et in range(EO):
                    pss = [psum.tile([P, FD], f32, tag="mm", name=f"ps{c}") for c in range(NQC)]
                    for skt, c in mm_order(SKT, NQC):
                        nc.tensor.matmul(
                            pss[c][:], V_s[:, skt, ts(et, P)], E_s[:, skt, ts(c, FD)],
                            start=(skt == 0), stop=(skt == SKT - 1),
                        )
                    for c in range(NQC):
                        ot = ostage.tile([P, FD], f32, tag="ot")
                        nc.vector.tensor_tensor(
                            ot[:], pss[c][:], rden[:, ts(c, FD)], mult
                        )
                        nc.sync.dma_start(outT[ts(et, P), ts(c, FD)], ot[:])

    if ldw_elide:
        n = _elide_redundant_ldweights(nc, mybir)
        print(f"ldweights elided: {n}")

    nc.compile()
    return nc


def get_nc():
    if "nc" not in _NC_CACHE:
        _NC_CACHE["nc"] = build_nc()
    return _NC_CACHE["nc"]


def make_in_maps(q, k, v, W_q, W_k, W_v, collective=True):
    bf = ml_dtypes.bfloat16
    wq = np.ascontiguousarray(W_q.astype(bf))
    wk = np.ascontiguousarray(W_k.astype(bf))
    wv = np.ascontiguousarray(W_v.astype(bf))
    kTb = [np.ascontiguousarray(k[b].astype(bf).T) for b in range(B)]
    vTb = [np.ascontiguousarray(v[b].astype(bf).T) for b in range(B)]
    in_maps = []
    for c in range(8):
        b, h = c // 2, c % 2
        qTc = np.ascontiguousarray(q[b, h * SQ:(h + 1) * SQ, :].astype(bf).T)
        kTc, vTc = kTb[b], vTb[b]
        if collective and h == 1:
            # odd core projects the second key-half: swap halves so its own
            # half sits in columns 0:1024 (the projected range)
            kTc = np.ascontiguousarray(
                np.concatenate([kTc[:, SKH:], kTc[:, :SKH]], axis=1))
            vTc = np.ascontiguousarray(
                np.concatenate([vTc[:, SKH:], vTc[:, :SKH]], axis=1))
        in_maps.append({
            "qT": qTc, "kT": kTc, "vT": vTc,
            "wq": wq, "wk": wk, "wv": wv,
        })
    return in_maps


def kernel(q, k, v, W_q, W_k, W_v):
    from concourse import bass_utils

    q, k, v = np.asarray(q), np.asarray(k), np.asarray(v)
    W_q, W_k, W_v = np.asarray(W_q), np.asarray(W_k), np.asarray(W_v)
    nc = get_nc()
    in_maps = make_in_maps(q, k, v, W_q, W_k, W_v)
    res = bass_utils.run_bass_kernel_spmd(nc, in_maps, core_ids=list(range(8)))
    out = np.empty((B, S, E), dtype=np.float32)
    for c in range(8):
        b, h = c // 2, c % 2
        out[b, h * SQ:(h + 1) * SQ, :] = res.results[c]["outT"].T
    return out
